# revision 2
# baseline (speedup 1.0000x reference)
"""Trainium2 Bass kernel v2 for nn_Action_37890201485804 (scatter_memory).

Pointer-generator head, restructured for DMA-roofline pacing:

  out_unnorm[(b,l), v] = exp(scale*gen_logit) + sum_{entries e -> v} exp(scale*copy_logit_e)
  out = out_unnorm / row_sum(out_unnorm)        (division on host)

Key structure vs the v1 kernel:
  * gen matmul in fp8e4 DoubleRow (2 K-subtiles per pass, 0.5 cyc/row)
    with host-side scaling (dec*8, W*32, exp scale folded); halves both
    the dominant gen_W DMA (16.8 -> 8.4 MB) and the PE time.
  * masked copy sources are dropped on the host (their softmax weight is
    exactly 0), shrinking the copy space to the unmasked entries only,
    and dup targets need no special handling (two one-hot rows add).
  * rows live on 64 partitions (8b+l): matmul time only depends on the
    moving free dim, so M=64 is free, and the packed fp16 output is a
    plain [64, 1024] DMA per tile pair (2.1 MB vs 8.4 MB f32 unpacked).
  * scatter: ONE ap_gather pulls, for every (b,l) partition, the copy
    probs of all 128 entries of each v-tile chunk (a single shared index
    list; wrong-batch values are garbage by construction).  Each chunk
    is PE-transposed to entry-major and a fused DVE mask*cast applies
    the block-diagonal batch mask; an iota==key one-hot then routes
    entries to vocab columns via one fp16 matmul per chunk.
  * no softmax barrier on device: all copy mass lands inside V, so the
    denominator equals the row sum of the unnormalized output, computed
    on the host after the fp16 result is gathered.

Per-core DMA ~13 MB => ~36 us roofline at ~360 GB/s.
"""

import sys

sys.path.insert(0, "/opt/trn_rl_repo")

import numpy as np
import ml_dtypes

F8 = ml_dtypes.float8_e4m3

import concourse.bass as bass  # noqa: F401
import concourse.tile as tile
from concourse import bacc, mybir, library_config
from concourse.bass_utils import run_bass_kernel_spmd

# ---------------------------------------------------------------- constants
B, LA, H, V = 64, 8, 512, 16384
PREF, PROF, STATE, CTX, REL = 10, 10, 10, 256, 30
S = PREF + PROF + STATE + CTX + REL  # 316
SCALE = float(H) ** -0.5

NCORE = 8
BL = B // NCORE          # local batches per core (8)
TS = 512                 # v-tile width
NT = V // TS             # 32 v-tiles
TG = 2                   # v-tiles per gen_W DMA group
SKEW = 12                # gen-tiles of lead before scatter work starts
SW = 8                   # dec scale for fp8
WW = 32.0                # gen_W scale for fp8
F32 = mybir.dt.float32
FP16 = mybir.dt.float16
FP8 = mybir.dt.float8e4
I16 = mybir.dt.int16

_BUILD_CACHE: dict = {}


# ================================================================ builder
def _build(csw: int, chunks_t: tuple, genb_nz: bool, debug: str = ""):
    """Build + compile the 8-core SPMD graph.

    csw: copy-space width (multiple of 128; unmasked entries per batch).
    chunks_t: per v-tile number of 128-entry scatter chunks.
    """
    key = (csw, chunks_t, genb_nz, debug)
    if key in _BUILD_CACHE:
        return _BUILD_CACHE[key]

    NCH = int(sum(chunks_t))
    NIDX = NCH * 128

    # per tile the chunk ids
    tile_chunks = []
    nch = 0
    for t in range(NT):
        tile_chunks.append(list(range(nch, nch + chunks_t[t])))
        nch += chunks_t[t]

    nc = bacc.Bacc(
        "TRN2", target_bir_lowering=False, debug=False, num_devices=NCORE,
    )

    def din(name, shape, dtype=F32):
        return nc.dram_tensor(name, list(shape), dtype, kind="ExternalInput").ap()

    genw = din("genw", (128, 4, V), FP8)          # W[128s+p, v]*WW
    decq = din("decq", (128, 4, 64), FP8)         # dec^T*SW, (b,l8) cols
    dec_cl = din("dec_cl", (128, 32 * 64), FP16)  # block-diag dec chunks
    src_cs = din("src_cs", (BL * H, csw), FP16)   # src^T, unmasked sorted entries
    ident64_in = din("ident64_in", (64, 64), FP16)
    iota_in = din("iota_in", (128, TS), FP16)     # 0..511 per partition
    bmask_in = din("bmask_in", (128, 64), FP16)   # block-diag batch mask
    NI16 = NCH * 16                               # d=8 block indices
    gidx_in = din("gidx_in", (64, max(NI16 // 16, 1)), I16)  # 16-wrapped blocks
    tw_in = din("tw_in", (128, max(NCH, 1)), F32)            # one-hot keys
    if genb_nz:
        ebias_in = din("ebias_in", (1, V), FP16)  # exp(scale*gen_b)
    out = nc.dram_tensor("out", [BL * LA, V], FP16, kind="ExternalOutput").ap()

    with tile.TileContext(nc) as tc:
        with (
            tc.tile_pool(name="const", bufs=1) as constp,
            tc.tile_pool(name="srcs", bufs=2) as srcp,
            tc.tile_pool(name="pcbp", bufs=1) as pcbp,
            tc.tile_pool(name="gathp", bufs=1) as gathp,
            tc.tile_pool(name="genw", bufs=3) as genwp,
            tc.tile_pool(name="pgen", bufs=SKEW + 4) as pgenp,
            tc.tile_pool(name="ptcm", bufs=6) as ptcmp,
            tc.tile_pool(name="ohs", bufs=24) as ohsp,
            tc.tile_pool(name="outs", bufs=3) as outp,
            tc.tile_pool(name="gen_ps", bufs=2, space="PSUM") as genps,
            tc.tile_pool(name="cps_ps", bufs=1, space="PSUM") as cpsps,
            tc.tile_pool(name="tr_ps", bufs=2, space="PSUM") as trps,
            tc.tile_pool(name="cp_ps", bufs=3, space="PSUM") as cpps,
        ):
            # Q7 ucode library for ap_gather
            nc.gpsimd.load_library(library_config.ap_gather)

            # ---------------- small inputs
            # scalar queue: dec operands first (copy matmuls gate the front)
            decq_sb = constp.tile([128, 4, 64], FP8, tag="decq")
            nc.scalar.dma_start(decq_sb[:], decq[:, :, :])
            deccl_sb = constp.tile([128, 32 * 64], FP16, tag="deccl")
            nc.scalar.dma_start(deccl_sb[:], dec_cl[:, :])
            ident64_sb = constp.tile([64, 64], FP16, tag="ident64")
            nc.scalar.dma_start(ident64_sb[:], ident64_in[:, :])
            if genb_nz:
                ebias_sb = constp.tile([1, V], FP16, tag="ebias")
                nc.scalar.dma_start(ebias_sb[:], ebias_in[:, :])

            # src tiles: one 3D-AP DMA per batch ([512, csw] -> [128, 4, csw]),
            # interleaved with the first genw groups across both queues
            src_sb = {}
            wts = [genwp.tile([128, 4, TG * TS], FP8, tag="wt", name="wt")
                   for _ in range(3)]

            def dma_src(bq, eng):
                st = srcp.tile([128, 4, csw], FP16, tag=f"s{bq}", name="st")
                eng.dma_start(
                    st[:],
                    src_cs[bq * H:(bq + 1) * H, :].rearrange(
                        "(k p) c -> p k c", k=4))
                src_sb[bq] = st

            nc.sync.dma_start(wts[0][:], genw[:, :, 0:TG * TS])
            dma_src(0, nc.sync)
            nc.sync.dma_start(
                wts[1][:], genw[:, :, TG * TS:2 * TG * TS])
            dma_src(1, nc.sync)
            nc.sync.dma_start(
                wts[2][:], genw[:, :, 2 * TG * TS:3 * TG * TS])
            dma_src(2, nc.sync)
            dma_src(3, nc.sync)
            for bq in range(4, BL):
                dma_src(bq, nc.scalar)

            # scalar queue: scatter metadata (needed by ~10us)
            iota_sb = constp.tile([128, TS], FP16, tag="iota")
            nc.scalar.dma_start(iota_sb[:], iota_in[:, :])
            bmask_sb = constp.tile([128, 64], FP16, tag="bmask")
            nc.scalar.dma_start(bmask_sb[:], bmask_in[:, :])
            gidx_sb = constp.tile([64, max(NI16 // 16, 1)], I16, tag="gidx")
            nc.scalar.dma_start(gidx_sb[:], gidx_in[:, :])
            tw_sb = constp.tile([128, max(NCH, 1)], F32, tag="tw")
            nc.scalar.dma_start(tw_sb[:], tw_in[:, :])

            # ---------------- copy phase state (emitted inside the
            # main loop, interleaved with the gen stream)
            cps = cpsps.tile([64, csw], F32)
            border = [0, 4, 1, 5, 2, 6, 3, 7]   # DMA arrival order
            pcb = pcbp.tile([64, csw], FP16, tag="pcb")
            gath = gathp.tile([64, max(NIDX, 4)], FP16, tag="gath")

            def emit_copy(j):
                bq, kc = border[j // 4], j % 4
                nc.tensor.matmul(
                    cps[:, :],
                    deccl_sb[:, 64 * (4 * bq + kc):64 * (4 * bq + kc) + 64],
                    src_sb[bq][:, kc, :],
                    start=(j == 0), stop=(j == 31),
                )

            def emit_gather():
                nc.scalar.activation(
                    pcb[:], cps[:], mybir.ActivationFunctionType.Exp,
                    scale=SCALE)
                # 8 split ap_gather calls so scatter consumption pipelines
                if debug == "nogather":
                    nc.vector.memset(gath[:], 0.0)
                elif NCH > 0:
                    bounds = [round(k * NCH / 8) for k in range(9)]
                    for k in range(8):
                        c0, c1 = bounds[k], bounds[k + 1]
                        if c0 == c1:
                            continue
                        nc.gpsimd.ap_gather(
                            gath[:, 128 * c0:128 * c1], pcb[:],
                            gidx_sb[:, c0:c1],
                            channels=64, num_elems=csw // 8, d=8,
                            num_idxs=16 * (c1 - c0))

            # ---------------- main loop: gen stream + skewed scatter.
            # Tiles are processed in PAIRS (even->PSUM rows 0:64, odd->
            # rows 64:128): two M=64 matmuls in opposite column groups run
            # concurrently on the PE.
            pgen_t = [None] * NT
            oh_tiles = {}

            def build_oh(c):
                o = ohsp.tile([128, TS], FP16, tag="oh", name="oh")
                nc.vector.tensor_scalar(
                    o[:], iota_sb[:], tw_sb[:, c:c + 1], None,
                    mybir.AluOpType.is_equal)
                oh_tiles[c] = o

            def emit_gen_pair(te):
                g = te // TG
                wt = wts[g % 3]
                # fp8 DoubleRow requires dst partition base 0, so the
                # two tiles use separate [64, TS] PSUM tiles
                pss = []
                for half in range(2):
                    tt = (te + half) % TG
                    ps = genps.tile([64, TS], F32, tag="gen", name="ps")
                    for pr in range(2):
                        nc.tensor.matmul(
                            ps[:],
                            decq_sb[:, 2 * pr:2 * pr + 2, :],
                            wt[:, 2 * pr:2 * pr + 2, TS * tt:TS * (tt + 1)],
                            start=(pr == 0), stop=(pr == 1),
                            perf_mode=mybir.MatmulPerfMode.DoubleRow,
                        )
                    pss.append(ps)
                if g + 3 < NT // TG:
                    wts[g % 3] = genwp.tile(
                        [128, 4, TG * TS], FP8, tag="wt", name="wt")
                    nc.sync.dma_start(
                        wts[g % 3][:],
                        genw[:, :, TG * TS * (g + 3):TG * TS * (g + 4)])
                for half in range(2):
                    t = te + half
                    pg = pgenp.tile([64, TS], FP16, tag="pg", name="pg")
                    nc.scalar.activation(
                        pg[:], pss[half][:],
                        mybir.ActivationFunctionType.Exp,
                        scale=SCALE / (SW * WW))
                    if genb_nz:
                        nc.vector.tensor_tensor(
                            pg[:], pg[:],
                            ebias_sb[0:1, TS * t:TS * (t + 1)
                                     ].partition_broadcast(64),
                            op=mybir.AluOpType.mult)
                    pgen_t[t] = pg

            def emit_scatter_pair(ue):
                cp2 = cpps.tile([128, TS], F32, tag="cp")
                halves = []
                for half in range(2):
                    u = ue + half
                    ids = tile_chunks[u]
                    pms = []
                    for c in ids:
                        trp = trps.tile([128, 64], F32, tag="tr", name="trp")
                        nc.tensor.matmul(
                            trp[:], gath[:, 128 * c:128 * (c + 1)],
                            ident64_sb[:], start=True, stop=True)
                        pm = ptcmp.tile([128, 64], FP16, tag="pm", name="pm")
                        nc.vector.tensor_tensor(
                            pm[:], trp[:], bmask_sb[:],
                            op=mybir.AluOpType.mult)
                        if c not in oh_tiles:
                            build_oh(c)
                        pms.append((c, pm))
                    halves.append(pms)
                # sequential accumulation groups per half (h0 then h64);
                # adjacent column-group instructions still overlap on the PE
                for half in range(2):
                    for j, (c, pm) in enumerate(halves[half]):
                        nc.tensor.matmul(
                            cp2[64 * half:64 * (half + 1), :],
                            pm[:], oh_tiles.pop(c)[:],
                            start=(j == 0),
                            stop=(j == len(halves[half]) - 1),
                        )
                ot = outp.tile([64, 2 * TS], FP16, tag="ot", name="ot")
                for half in range(2):
                    u = ue + half
                    sl = ot[:, TS * half:TS * (half + 1)]
                    cph = cp2[64 * half:64 * (half + 1), :]
                    if debug == "pg" or not tile_chunks[u]:
                        nc.vector.tensor_copy(sl, pgen_t[u][:])
                    elif debug == "cp":
                        nc.vector.tensor_copy(sl, cph)
                    else:
                        nc.vector.scalar_tensor_tensor(
                            sl, cph, 1.0, pgen_t[u][:],
                            op0=mybir.AluOpType.mult, op1=mybir.AluOpType.add)
                    pgen_t[u] = None
                nc.gpsimd.dma_start(
                    out[:, TS * ue:TS * (ue + 2)], ot[:, :])

            for te in range(0, NT, 2):
                emit_gen_pair(te)
                if te < 8:
                    for j in range(8 * (te // 2), 8 * (te // 2) + 8):
                        emit_copy(j)
                if te == 8:
                    emit_gather()
                    # prebuild trailing one-hots in the DVE's idle window
                    for t in range(16, NT):
                        for c in tile_chunks[t]:
                            build_oh(c)
                if te >= SKEW:
                    emit_scatter_pair(te - SKEW)
            for ue in range(NT - SKEW, NT, 2):
                emit_scatter_pair(ue)

    nc.compile()
    _BUILD_CACHE[key] = nc
    return nc


# ================================================================ host prep
def _onehot_idx(mat):
    """Return [B, p] argmax indices if mat rows are exact one-hot, else None."""
    mat = np.asarray(mat)
    idx = mat.argmax(-1)
    if not (np.take_along_axis(mat, idx[..., None], -1) == 1.0).all():
        return None
    if (mat != 0).sum(-1).max() != 1:
        return None
    return idx.astype(np.int64)


def _prep(dec_out, src_hidden, src_mask, pv_m, l, tp, related,
          gen_W, gen_b, context, glo2loc):
    f32 = np.float32
    dec_out = np.asarray(dec_out, f32)
    src_hidden = np.asarray(src_hidden, f32)
    src_mask = np.asarray(src_mask)
    gen_W = np.asarray(gen_W, f32)
    gen_b = np.asarray(gen_b, f32)
    context = np.asarray(context)
    glo2loc = np.asarray(glo2loc)

    oh = [_onehot_idx(m) for m in (pv_m, l, tp, related)]
    if any(o is None for o in oh):
        return None  # dense fallback handled by caller

    genb_nz = bool(np.any(gen_b != 0.0))

    transfer = glo2loc[context].astype(np.int64)               # [B, CTX]
    fr = np.concatenate([np.arange(30), 286 + np.arange(30)])
    targets = np.concatenate(
        [oh[0], oh[1], oh[2], transfer, oh[3]], 1)             # [B, 316]
    srcrow = np.concatenate(
        [np.tile(fr[:30], (B, 1)), 30 + np.tile(np.arange(CTX), (B, 1)),
         np.tile(fr[30:], (B, 1))], 1)                         # [B, 316]
    m = src_mask[:, 0, :]                                      # [B, S]
    keep = np.take_along_axis(m, srcrow, 1) == 1               # [B, 316]

    # per-batch unmasked entries sorted by target
    tgt_b, srw_b, K_b = [], [], np.zeros(B, np.int64)
    for b in range(B):
        tb = targets[b][keep[b]]
        sb = srcrow[b][keep[b]]
        o = np.argsort(tb, kind="stable")
        tgt_b.append(tb[o]); srw_b.append(sb[o]); K_b[b] = len(tb)

    # per (batch, tile) runs, padded to 8 (ap_gather d=8 blocks), and
    # the SPMD-uniform chunk structure
    lo = np.zeros((B, NT), np.int64)
    hi = np.zeros((B, NT), np.int64)
    for b in range(B):
        bounds = np.searchsorted(tgt_b[b], np.arange(NT + 1) * TS)
        lo[b], hi[b] = bounds[:-1], bounds[1:]
    cnt = hi - lo
    L = ((cnt + 7) // 8) * 8                                   # padded runs
    colstart = np.concatenate(
        [np.zeros((B, 1), np.int64), np.cumsum(L, 1)[:, :-1]], 1)
    csw = int(np.ceil((colstart[:, -1] + L[:, -1]).max() / 64) * 64)
    assert csw <= 512, f"padded copy space {csw} > 512"
    chunks_t = tuple(
        int(np.ceil(cnt[:, t].max() / 16)) if cnt[:, t].max() > 0 else 0
        for t in range(NT))
    NCH = int(sum(chunks_t))
    NIDX = NCH * 128
    NI16 = NCH * 16

    # src^T in padded copy-space order (all cores at once)
    src_cs = np.zeros((B, H, csw), np.float16)
    for b in range(B):
        for t in range(NT):
            n = cnt[b, t]
            if n:
                e = slice(lo[b, t], hi[b, t])
                cs = colstart[b, t]
                src_cs[b, :, cs:cs + n] = src_hidden[b, srw_b[b][e], :].T

    iota_in = np.tile(np.arange(TS, dtype=np.float16), (128, 1))
    ident64 = np.eye(64, dtype=np.float16)
    bmask = (np.arange(128)[:, None] // 16
             == np.arange(64)[None, :] // 8).astype(np.float16)

    genw = np.ascontiguousarray(
        np.clip(gen_W.reshape(4, 128, V).transpose(1, 0, 2) * WW,
                -240, 240).astype(F8))
    if genb_nz:
        eb = np.exp(SCALE * gen_b).astype(np.float16)[None, :]

    in_maps = []
    for c in range(NCORE):
        gbi = np.arange(c * BL, (c + 1) * BL)
        d = dec_out[c * BL:(c + 1) * BL]                       # [BL, LA, H]

        decq = np.zeros((128, 4, 64), f32)
        for b in range(BL):
            for s in range(4):
                decq[:, s, 8 * b:8 * b + LA] = d[b].T[128 * s:128 * (s + 1)] * SW

        dec_cl = np.zeros((128, 32 * 64), f32)
        for j in range(32):
            bq, kc = j // 4, j % 4
            hs = slice(128 * kc, 128 * (kc + 1))
            dec_cl[:, 64 * j + 8 * bq:64 * j + 8 * bq + LA] = d[bq].T[hs]

        # gather block ids + one-hot keys, uniform chunk structure.
        # Chunk ci (tile t, sub i) holds, per batch b, padded run slots
        # [16i, 16i+16) as two d=8 blocks at idx positions 16ci+2b+h.
        # One SHARED list serves every partition (wrong-batch reads are
        # garbage, masked after the transpose).
        gflat = np.zeros(max(NI16, 1), np.int16)
        tw = np.full((128, max(NCH, 1)), -1.0, f32)
        ci = 0
        for t in range(NT):
            for i in range(chunks_t[t]):
                for b in range(BL):
                    gb = gbi[b]
                    for h in range(2):
                        s0 = 16 * i + 8 * h
                        if s0 < L[gb, t]:
                            gflat[16 * ci + 2 * b + h] =                                 colstart[gb, t] // 8 + 2 * i + h
                        n = min(max(cnt[gb, t] - s0, 0), 8)
                        for r in range(int(n)):
                            tw[16 * b + 8 * h + r, ci] =                                 tgt_b[gb][lo[gb, t] + s0 + r] - TS * t
                ci += 1
        gidx = np.zeros((64, max(NI16 // 16, 1)), np.int16)
        ii = np.arange(NI16)
        for g in range(4):
            gidx[16 * g + (ii % 16), ii // 16] = gflat[:NI16]
        im = dict(
            genw=genw,
            decq=np.ascontiguousarray(np.clip(decq, -240, 240).astype(F8)),
            dec_cl=np.ascontiguousarray(dec_cl.astype(np.float16)),
            src_cs=np.ascontiguousarray(
                src_cs[c * BL:(c + 1) * BL].reshape(BL * H, csw)),
            ident64_in=ident64,
            iota_in=iota_in,
            bmask_in=bmask,
            gidx_in=gidx,
            tw_in=np.ascontiguousarray(tw),
        )
        if genb_nz:
            im["ebias_in"] = eb
        in_maps.append(im)
    return in_maps, csw, chunks_t, genb_nz


def _fallback(dec_out, src_hidden, src_mask, pv_m, l, tp, related,
              gen_W, gen_b, context, glo2loc):
    """Pure numpy reference (used only if the one-hot fast path fails)."""
    f = np.float32
    NEG = -1e9
    dec_out = np.asarray(dec_out, f)
    gen_logit = np.einsum('bld,dv->blv', dec_out, np.asarray(gen_W, f))
    gen_logit = gen_logit + np.asarray(gen_b, f)
    copy_logit = np.einsum('bld,bsd->bls', dec_out, np.asarray(src_hidden, f))
    copy_logit = np.where(np.asarray(src_mask) == 0, NEG, copy_logit)
    logits = np.concatenate([gen_logit, copy_logit], -1) * SCALE
    e = np.exp(logits - logits.max(-1, keepdims=True))
    probs = e / e.sum(-1, keepdims=True)
    gen_p = probs[..., :V]
    o = V
    m_p = np.einsum('blp,bpv->blv', probs[..., o:o + PREF], np.asarray(pv_m, f)); o += PREF
    l_p = np.einsum('blp,bpv->blv', probs[..., o:o + PROF], np.asarray(l, f)); o += PROF
    tp_p = np.einsum('blp,bpv->blv', probs[..., o:o + STATE], np.asarray(tp, f)); o += STATE
    ctx_p_raw = probs[..., o:o + CTX]; o += CTX
    rel_p = np.einsum('blp,bpv->blv', probs[..., o:], np.asarray(related, f))
    tr = np.asarray(glo2loc)[np.asarray(context)]
    ctx_p = np.zeros((B, LA, V), f)
    for b in range(B):
        np.add.at(ctx_p[b], (slice(None), tr[b]), ctx_p_raw[b])
    return gen_p + l_p + tp_p + ctx_p + rel_p + m_p


# ================================================================ entry
def kernel(**inputs) -> np.ndarray:
    prep = _prep(**inputs)
    if prep is None:
        return _fallback(**inputs)
    in_maps, csw, chunks_t, genb_nz = prep
    nc = _build(csw, chunks_t, genb_nz)
    res = run_bass_kernel_spmd(nc, in_maps, core_ids=list(range(NCORE)))
    outs = []
    for c in range(NCORE):
        o = np.asarray(res.results[c]["out"]).astype(np.float32)
        o = o.reshape(BL, LA, V)
        o /= o.sum(-1, keepdims=True)
        outs.append(o)
    return np.concatenate(outs, 0)
